# revision 5
# baseline (speedup 1.0000x reference)
"""Embedding-lookup (GatherNd, batch_dims=1) Trainium2 kernel.

Problem: data [8, 50000, 256] f32, indices [8, 65536, 1] int -> out [8, 65536, 256] f32
  out[b, j, :] = data[b, indices[b, j, 0], :]

Sharding: one batch element per NeuronCore (8 cores, data parallel, no
cross-core communication).

Per-core kernel: chunked indirect-DMA gather.  Each chunk gathers
128 partitions x K rows (1 KB each) from the HBM table into an SBUF tile
via one SWDGE indirect DMA (descriptor generation is 16-lane vectorized
on the Q7), then writes the tile back to HBM with a contiguous
32 KB-per-partition HWDGE DMA.  Chunks are pipelined through a
3-deep tile pool, so gathers and writebacks overlap.

Indices arrive as int64 (or int32).  int64 is handed to the device as a
bit-identical int32-pair view (numpy .view, zero host compute); the
low words are extracted on-device with a stride-2 DVE copy.
"""

import os
import sys

for _p in ("/opt/trn_rl_repo", "/opt/pypackages"):
    if _p not in sys.path:
        sys.path.append(_p)

import numpy as np

from concourse import bacc, bass, mybir, tile
from concourse.bass_utils import run_bass_kernel_spmd


def _ensure_axon_ntff_hook():
    """Provide antenv.axon_hooks if the image lacks it, so trace=True under
    axon works (bass_utils imports it unconditionally when tracing)."""
    try:
        import antenv.axon_hooks  # noqa: F401

        return
    except ImportError:
        pass
    import contextlib
    import ctypes
    import types

    mod = types.ModuleType("antenv.axon_hooks")
    _holder = {}

    def _build_hook(so_path="/opt/axon/libaxon_pjrt.so"):
        if not os.path.exists(so_path):
            return None
        lib = ctypes.CDLL(so_path)
        if not hasattr(lib, "axon_start_nrt_profile"):
            return None
        lib.axon_start_nrt_profile.argtypes = [
            ctypes.POINTER(ctypes.c_int64),
            ctypes.c_size_t,
        ]
        lib.axon_start_nrt_profile.restype = ctypes.c_int64
        lib.axon_stop_nrt_profile.argtypes = [ctypes.c_char_p]
        lib.axon_stop_nrt_profile.restype = ctypes.c_int64

        @contextlib.contextmanager
        def _hook(output_dir, device_ids):
            import jax

            jax.devices()
            if device_ids:
                ids = (ctypes.c_int64 * len(device_ids))(*device_ids)
                rc = lib.axon_start_nrt_profile(ids, len(device_ids))
            else:
                rc = lib.axon_start_nrt_profile(None, 0)
            if rc != 0:
                raise RuntimeError(f"axon_start_nrt_profile rc={rc}")
            try:
                yield
            finally:
                n = lib.axon_stop_nrt_profile(str(output_dir).encode())
                if n <= 0:
                    print(f"ntff profile: {n} files written to {output_dir}")

        return _hook

    def set_axon_ntff_profile_hook(h):
        _holder["h"] = h

    def get_axon_ntff_profile_hook():
        if "h" not in _holder:
            _holder["h"] = _build_hook()
        return _holder["h"]

    mod.set_axon_ntff_profile_hook = set_axon_ntff_profile_hook
    mod.get_axon_ntff_profile_hook = get_axon_ntff_profile_hook
    sys.modules["antenv.axon_hooks"] = mod
    try:
        import antenv

        antenv.axon_hooks = mod
    except ImportError:
        pass


_ensure_axon_ntff_hook()

B, N, D, M = 8, 50000, 256, 65536
P = 128                    # SBUF partitions
K = 32                     # gathered rows per partition per chunk
CHUNK = P * K              # 4096 rows per chunk
NCHUNK = M // CHUNK        # 16
PAD = 8                    # f32 pad per gathered row: keeps rows as separate
                           # DMA descriptor runs (32B-aligned stride)

_cache = {}
last_results = None        # BassKernelResults of the most recent run (for test.py)


def build_nc(idx64: bool):
    """Build + compile the per-core SPMD program.

    idx64: whether the index dram tensor is an int32-pair view of int64
    (size 2*M) or plain int32 (size M).
    """
    nc = bacc.Bacc(
        "TRN2", target_bir_lowering=False, debug=False, enable_asserts=False
    )
    data = nc.dram_tensor("data", [N, D], mybir.dt.float32, kind="ExternalInput")
    idx_words = (2 if idx64 else 1) * M
    idx = nc.dram_tensor("idx", [idx_words], mybir.dt.int32, kind="ExternalInput")
    out = nc.dram_tensor("out", [M, D], mybir.dt.float32, kind="ExternalOutput")

    c = idx_words // M * K  # int32 words per (partition, chunk) run
    with tile.TileContext(nc) as tc:
        with (
            tc.tile_pool(name="ipool", bufs=1) as ipool,
            tc.tile_pool(name="gpool", bufs=3) as gpool,
        ):
            # Load all indices: partition p gets, for every chunk i, the
            # c contiguous words holding indices j = i*CHUNK + p*K + [0..K).
            ip = ipool.tile([P, NCHUNK * c], mybir.dt.int32)
            nc.sync.dma_start(
                out=ip[:].rearrange("p (i c) -> p i c", i=NCHUNK),
                in_=idx[:].rearrange("(i p c) -> p i c", i=NCHUNK, p=P),
            )
            if idx64:
                # keep the low int32 of each little-endian int64 pair
                idx_all = ipool.tile([P, NCHUNK * K], mybir.dt.int32)
                nc.vector.tensor_copy(out=idx_all[:], in_=ip[:, 0 : NCHUNK * c : 2])
            else:
                idx_all = ip

            # out viewed so chunk i / partition p is one contiguous K*D run
            out_r = out[:].rearrange("(i p c) d -> i p (c d)", i=NCHUNK, p=P)
            for i in range(NCHUNK):
                # The SWDGE consumes one index per DMA descriptor (a
                # contiguous destination run), so each gather instruction
                # moves exactly one row per partition: offsets [P, 1],
                # destination [P, D].  This is the production-proven
                # scatter_add shape; K of them fill one writeback tile.
                g = gpool.tile([P, K * D], mybir.dt.float32)
                for c in range(K):
                    nc.gpsimd.indirect_dma_start(
                        out=g[:, c * D : (c + 1) * D],
                        out_offset=None,
                        in_=data[:],
                        in_offset=bass.IndirectOffsetOnAxis(
                            ap=idx_all[:, i * K + c : i * K + c + 1], axis=0
                        ),
                    )
                nc.sync.dma_start(out=out_r[i], in_=g[:])
    nc.compile()
    return nc


def _idx_words(indices_b: np.ndarray) -> np.ndarray:
    """Flat int32 word view of one batch's indices (no host compute for int64)."""
    a = np.ascontiguousarray(indices_b).reshape(-1)
    if a.dtype == np.int64:
        return a.view(np.int32)
    if a.dtype == np.int32:
        return a
    # unexpected dtype (e.g. uint32) - normalize via cheap cast
    return a.astype(np.int32)


def kernel(data, indices) -> np.ndarray:
    global last_results
    data = np.asarray(data)
    indices = np.asarray(indices)
    assert data.shape == (B, N, D) and indices.shape[:2] == (B, M)

    idx64 = indices.dtype == np.int64
    if idx64 not in _cache:
        _cache[idx64] = build_nc(idx64)
    nc = _cache[idx64]

    in_maps = [
        {
            "data": np.ascontiguousarray(data[b], dtype=np.float32),
            "idx": _idx_words(indices[b]),
        }
        for b in range(B)
    ]
    res = run_bass_kernel_spmd(nc, in_maps, list(range(B)))
    last_results = res
    return np.stack([res.results[b]["out"] for b in range(B)], axis=0)
